# revision 6
# baseline (speedup 1.0000x reference)
"""Trainium2 Bass kernel for nn_Conv2dShareQ (vq_codebook) — 1D Winograd F(2,3).

Sharding: 4-way batch x 2-way conv group per core (as before).

The 3x3 conv uses Winograd F(2,3) along W only (direct along H and input
channels), cutting PE streamed columns to 2/3 of direct:

  per (ic, row r) and output column pair q:  d_k = xpad[ic, r, 2q+k]
      V0 = d0-d2, V1 = d1+d2, V2 = d2-d1, V3 = d1-d3        (input transform)
  per (oc, ic, ky), g_k = w[oc, ic, ky, k]:
      U0 = g0, U1 = (g0+g1+g2)/2, U2 = (g0-g1+g2)/2, U3 = g2 (weight transform)
  M_xi[oc, h, q] = sum_{ic, ky} U_xi[oc, ic, ky] * V_xi[ic, h+ky, q]   (PE)
  y[oc, h, 2q]   = M0 + M1 + M2      (bias folded into M1's eviction)
  y[oc, h, 2q+1] = M1 - M2 - M3

Engine placement: gather + U on DVE; V on GPSIMD (de-interleaved host layout
makes all four planes plain tensor_tensor ops); M on PE (4 PSUM banks per
(im, mt, h-tile) set, 6 matmuls each); M eviction on ACT (PSUM -> bf16 SBUF);
y assembly 4 tensor_tensor per set on DVE (GPSIMD for the last image).
Output bf16, upcast on host.
"""

import sys

for _p in ("/opt/trn_rl_repo", "/root/.axon_site/_ro/trn_rl_repo"):
    if _p not in sys.path:
        sys.path.append(_p)

import numpy as np
import ml_dtypes

import concourse.bass as bass
import concourse.mybir as mybir
from concourse.tile import TileContext, ScopedClock
from concourse.tile_scheduler import N_PROCS
from bass_rust import VectorClock
from concourse.bass_utils import run_bass_kernel_spmd

F32 = mybir.dt.float32
BF16 = mybir.dt.bfloat16
I8 = mybir.dt.int8

N_IMG_PER_CORE = 4
N_KT = 2
N_MT = 2
N_OFF = 9
H = W = 56
HP = 58                       # padded rows
Q = W // 2                    # 28 output column pairs
QP = Q + 1                    # 29 de-interleaved columns per parity
HW = H * W
N_CENT = 16
CH = N_OFF * 128              # 1152
LAB_FREE = N_MT * CH          # 2304
X_FREE = 2 * HP * QP          # 3364
HT_SIZES = (16, 16, 16, 8)    # h-tiles per (im, mt): 4 PSUM banks each


class SplitDrainTileContext(TileContext):
    """Tail drain split one proc per drain: this walrus build rejects CTRL
    instructions carrying more than one sem wait."""

    def _drain_and_barrier(self, tick_clock, wait_clock):
        gc = tick_clock.global_clock
        for p in range(N_PROCS):
            t = gc[p]
            if t <= 0:
                continue
            vec = [t if q == p else 0 for q in range(N_PROCS)]
            d = self.nc.sync.drain()
            wait_clock.add_sem_waits(d.ins, ScopedClock({None: VectorClock(vec)}))
        self.nc.all_engine_barrier()
        assert self.sems is not None
        popped = self.nc._tile_sem_poison_stack.pop()
        assert popped is self._sem_poison
        self.nc.clear_and_free_semaphores(list(self.sems.allocated().values()))
        self.nc.all_engine_barrier()


def _split_multi_waits(nc, limit=1):
    """Hoist excess sem waits onto wait-only EventSemaphore instructions."""
    for f in nc.m.functions:
        for bb in f.blocks:
            out = []
            for ins in bb.instructions:
                si = ins.sync_info
                if si is not None and si.on_wait and len(si.on_wait) > limit:
                    waits = list(si.on_wait)
                    for w in waits[:-limit]:
                        es = mybir.InstEventSemaphore(
                            name=f"waitsplit_{nc.next_id()}", ins=[], outs=[])
                        es.engine = ins.engine
                        es.sync_info = mybir.SyncInfo(on_wait=[w], on_update=[])
                        out.append(es)
                    si.on_wait = waits[-limit:]
                out.append(ins)
            bb.instructions[:] = out


def build_program():
    nc = bass.Bass()

    x_in = nc.dram_tensor("x", [N_IMG_PER_CORE, N_KT, 128, X_FREE], BF16,
                          kind="ExternalInput")
    labels_in = nc.dram_tensor("labels", [N_KT, 128, LAB_FREE], I8,
                               kind="ExternalInput")
    cent_in = nc.dram_tensor("centroids", [N_CENT], F32, kind="ExternalInput")
    bias_in = nc.dram_tensor("bias", [N_MT, 128], F32, kind="ExternalInput")
    out = nc.dram_tensor("out", [N_IMG_PER_CORE, N_MT, 128, HW], BF16,
                         kind="ExternalOutput")

    with SplitDrainTileContext(nc) as tc:
        with (
            tc.tile_pool(name="consts", bufs=1) as consts,
            tc.tile_pool(name="lab_f", bufs=1) as lab_f_pool,
            tc.tile_pool(name="wq", bufs=1) as wq_pool,
            tc.tile_pool(name="u12", bufs=1) as u12_pool,
            tc.tile_pool(name="tbuf", bufs=1) as tbuf_pool,
            tc.tile_pool(name="xbuf", bufs=1) as xbuf_pool,
            tc.tile_pool(name="vbuf", bufs=1) as vbuf_pool,
            tc.tile_pool(name="msb", bufs=3) as msb_pool,
            tc.tile_pool(name="ytmp", bufs=3) as ytmp_pool,
            tc.tile_pool(name="lstage", bufs=2) as lstage_pool,
            tc.tile_pool(name="obuf", bufs=4) as obuf_pool,
            tc.tile_pool(name="psum", bufs=8, space="PSUM") as psum_pool,
        ):
            cent_sb = consts.tile([128, N_CENT], F32)
            cent_bcast = bass.AP(tensor=cent_in[:].tensor, offset=0,
                                 ap=[[0, 128], [1, N_CENT]])
            nc.sync.dma_start(out=cent_sb[:], in_=cent_bcast)

            lab_stage = {}

            def load_labels(mt):
                for kt in range(N_KT):
                    sl = slice(mt * CH, (mt + 1) * CH)
                    li = lstage_pool.tile([128, CH], I8, tag="ls",
                                          name=f"lab_st{mt}_{kt}")
                    nc.sync.dma_start(out=li[0:64, :], in_=labels_in[kt][0:64, sl])
                    nc.sync.dma_start(out=li[64:128, :], in_=labels_in[kt][64:128, sl])
                    lab_stage[(mt, kt)] = li

            load_labels(0)

            # x tiles: [128, 2(parity), 58, 29] bf16; im shares slots by parity
            xs = {}

            def load_x(im, kt):
                xs[(im, kt)] = xbuf_pool.tile(
                    [128, 2, HP, QP], BF16, tag=f"x{im % 2}_{kt}",
                    name=f"x{im}_{kt}")
                nc.sync.dma_start(out=xs[(im, kt)][:], in_=x_in[im, kt])

            load_x(0, 0)
            load_x(0, 1)

            bias_sb = consts.tile([128, N_MT], F32)
            for mt in range(N_MT):
                nc.sync.dma_start(out=bias_sb[:, mt:mt + 1], in_=bias_in[mt, :])

            load_labels(1)
            load_x(1, 0)
            load_x(1, 1)

            # ---- codebook gather (DVE) ----
            lab_f = [lab_f_pool.tile([128, LAB_FREE], BF16, tag=f"lf{kt}",
                                     name=f"lab_f{kt}")
                     for kt in range(N_KT)]
            # wq free layout per kt: [mt, ky, kx, oo]
            wq = [wq_pool.tile([128, N_MT, 3, 3, 128], BF16, tag=f"wq{kt}",
                               name=f"wq{kt}")
                  for kt in range(N_KT)]
            # u12 free layout per kt: [mt, ky, {u1,u2}, oo]
            u12 = [u12_pool.tile([128, N_MT, 3, 2, 128], BF16, tag=f"u{kt}",
                                 name=f"u12_{kt}")
                   for kt in range(N_KT)]

            def gather_chunk(mt, kt):
                sl = slice(mt * CH, (mt + 1) * CH)
                nc.vector.tensor_copy(out=lab_f[kt][:, sl],
                                      in_=lab_stage.pop((mt, kt))[:])
                t = tbuf_pool.tile([128, N_CENT, CH], BF16, tag="t", name="t")
                for v in range(N_CENT):
                    nc.vector.tensor_scalar(
                        out=t[:, v, :], in0=lab_f[kt][:, sl],
                        scalar1=float(v), scalar2=cent_sb[:, v:v + 1],
                        op0=mybir.AluOpType.is_equal,
                        op1=mybir.AluOpType.mult,
                    )
                s8 = tbuf_pool.tile([128, 8, CH], BF16, tag="s8", name="s8")
                s4 = tbuf_pool.tile([128, 4, CH], BF16, tag="s4", name="s4")
                s2 = tbuf_pool.tile([128, 2, CH], BF16, tag="s2", name="s2")
                wq_flat = wq[kt][:].rearrange("p a b c d -> p (a b c d)")
                hh = CH // 2
                for h in range(2):
                    hs = slice(h * hh, (h + 1) * hh)
                    nc.vector.tensor_tensor(out=s8[:, :, hs], in0=t[:, 0:8, hs],
                                            in1=t[:, 8:16, hs],
                                            op=mybir.AluOpType.add)
                for h in range(2):
                    hs = slice(h * hh, (h + 1) * hh)
                    nc.vector.tensor_tensor(out=s4[:, :, hs], in0=s8[:, 0:4, hs],
                                            in1=s8[:, 4:8, hs],
                                            op=mybir.AluOpType.add)
                for h in range(2):
                    hs = slice(h * hh, (h + 1) * hh)
                    nc.vector.tensor_tensor(out=s2[:, :, hs], in0=s4[:, 0:2, hs],
                                            in1=s4[:, 2:4, hs],
                                            op=mybir.AluOpType.add)
                for h in range(2):
                    hs = slice(h * hh, (h + 1) * hh)
                    nc.vector.tensor_tensor(
                        out=wq_flat[:, mt * CH + h * hh: mt * CH + (h + 1) * hh],
                        in0=s2[:, 0, hs], in1=s2[:, 1, hs],
                        op=mybir.AluOpType.add)

            def u_transform(mt, kt):
                """u1 = (g0+g1+g2)/2, u2 = (g0-g1+g2)/2 for all ky of (mt,kt)."""
                g0 = wq[kt][:, mt, :, 0, :]
                g1 = wq[kt][:, mt, :, 1, :]
                g2 = wq[kt][:, mt, :, 2, :]
                st = ytmp_pool.tile([128, 3, 3, 128], BF16, tag="ut", name="ut")
                nc.vector.tensor_tensor(out=st[:, 0], in0=g0, in1=g2,
                                        op=mybir.AluOpType.add)
                nc.vector.tensor_tensor(out=st[:, 1], in0=st[:, 0], in1=g1,
                                        op=mybir.AluOpType.add)
                nc.vector.tensor_tensor(out=st[:, 2], in0=st[:, 0], in1=g1,
                                        op=mybir.AluOpType.subtract)
                nc.vector.tensor_scalar(out=u12[kt][:, mt, :, 0, :],
                                        in0=st[:, 1], scalar1=0.5,
                                        scalar2=None,
                                        op0=mybir.AluOpType.mult,
                                        op1=mybir.AluOpType.bypass)
                nc.vector.tensor_scalar(out=u12[kt][:, mt, :, 1, :],
                                        in0=st[:, 2], scalar1=0.5,
                                        scalar2=None,
                                        op0=mybir.AluOpType.mult,
                                        op1=mybir.AluOpType.bypass)

            for kt in range(N_KT):
                gather_chunk(0, kt)
                u_transform(0, kt)

            # x for im2/im3 (slots shared by parity with im0/im1; DMAs wait
            # for the V transform of the earlier image to release the slot)
            load_x(2, 0)
            load_x(2, 1)
            load_x(3, 0)
            load_x(3, 1)

            # ---- V transform (GPSIMD), one xi-plane per op ----
            vtiles = {}

            def v_planes(im):
                for xi in range(4):
                    for kt in range(N_KT):
                        if (im, kt) not in vtiles:
                            vtiles[(im, kt)] = vbuf_pool.tile(
                                [128, 4, HP, Q], BF16, tag=f"v{im % 2}_{kt}",
                                name=f"v{im}_{kt}")
                        vt = vtiles[(im, kt)]
                        x = xs[(im, kt)]
                        d0 = x[:, 0, :, 0:Q]
                        d2 = x[:, 0, :, 1:Q + 1]
                        d1 = x[:, 1, :, 0:Q]
                        d3 = x[:, 1, :, 1:Q + 1]
                        if xi == 0:
                            nc.vector.tensor_tensor(out=vt[:, 0], in0=d0, in1=d2,
                                                    op=mybir.AluOpType.subtract)
                        elif xi == 1:
                            nc.vector.tensor_tensor(out=vt[:, 1], in0=d1, in1=d2,
                                                    op=mybir.AluOpType.add)
                        elif xi == 2:
                            nc.vector.tensor_tensor(out=vt[:, 2], in0=d2, in1=d1,
                                                    op=mybir.AluOpType.subtract)
                        else:
                            nc.vector.tensor_tensor(out=vt[:, 3], in0=d1, in1=d3,
                                                    op=mybir.AluOpType.subtract)

            # ---- M sets: PE + ACT eviction + y assembly ----
            def m_set(im, mt, ht):
                hr = HT_SIZES[ht]
                h0 = sum(HT_SIZES[:ht])
                n = hr * Q
                ms = [psum_pool.tile([128, HT_SIZES[0], Q], F32, tag="ps",
                                     name="ps") for _ in range(4)]
                for xi in range(4):
                    idx = 0
                    for kt in range(N_KT):
                        for ky in range(3):
                            if xi == 0:
                                lhsT = wq[kt][:, mt, ky, 0, :]
                            elif xi == 3:
                                lhsT = wq[kt][:, mt, ky, 2, :]
                            else:
                                lhsT = u12[kt][:, mt, ky, xi - 1, :]
                            rhs = vtiles[(im, kt)][:, xi, h0 + ky: h0 + ky + hr, :]
                            nc.tensor.matmul(ms[xi][:, 0:hr, :], lhsT, rhs,
                                             start=(idx == 0), stop=(idx == 5))
                            idx += 1
                mb = msb_pool.tile([128, 4, HT_SIZES[0] * Q], BF16, tag="mb",
                                   name="mb")
                for xi in range(4):
                    nc.scalar.activation(
                        out=mb[:, xi, 0:n],
                        in_=ms[xi][:, 0:hr, :].rearrange("p h q -> p (h q)"),
                        func=mybir.ActivationFunctionType.Identity,
                        bias=bias_sb[:, mt:mt + 1] if xi == 1 else 0.0,
                        scale=1.0,
                    )
                # y assembly: even = M0+M1+M2, odd = M1-M2-M3
                # stored as parity-major planes; host interleaves columns
                ob = obuf_pool.tile([128, 2, HT_SIZES[0] * Q], BF16, tag="ob",
                                    name="ob")
                yt = ytmp_pool.tile([128, 2, HT_SIZES[0] * Q], BF16, tag="yt",
                                    name="yt")
                nc.vector.tensor_tensor(out=yt[:, 0, 0:n], in0=mb[:, 0, 0:n],
                                        in1=mb[:, 1, 0:n],
                                        op=mybir.AluOpType.add)
                nc.vector.tensor_tensor(out=ob[:, 0, 0:n], in0=yt[:, 0, 0:n],
                                        in1=mb[:, 2, 0:n],
                                        op=mybir.AluOpType.add)
                nc.vector.tensor_tensor(out=yt[:, 1, 0:n], in0=mb[:, 1, 0:n],
                                        in1=mb[:, 2, 0:n],
                                        op=mybir.AluOpType.subtract)
                nc.vector.tensor_tensor(out=ob[:, 1, 0:n], in0=yt[:, 1, 0:n],
                                        in1=mb[:, 3, 0:n],
                                        op=mybir.AluOpType.subtract)
                # out free layout per (im, mt): [parity(2), 56h, 28q]
                dst = bass.AP(
                    tensor=out[im, mt].tensor,
                    offset=out[im, mt].offset + h0 * Q,
                    ap=[out[im, mt].ap[0], [H * Q, 2], [1, n]])
                nc.sync.dma_start(out=dst, in_=ob[:, :, 0:n])

            def block(im, mt):
                for ht in range(4):
                    m_set(im, mt, ht)

            v_planes(0)
            v_planes(1)
            for kt in range(N_KT):
                gather_chunk(1, kt)
                u_transform(1, kt)
            block(0, 0)
            block(1, 0)
            block(0, 1)
            v_planes(2)     # reuses im0's slots (released after block(0, 1))
            block(1, 1)
            v_planes(3)     # reuses im1's slots
            block(2, 0)
            block(2, 1)
            block(3, 0)
            block(3, 1)

    _split_multi_waits(nc)
    return nc


_NC_CACHE = None


def _get_nc():
    global _NC_CACHE
    if _NC_CACHE is None:
        _NC_CACHE = build_program()
    return _NC_CACHE


def make_in_maps(x, centroids, labels, bias):
    """Shard full inputs into 8 per-core input maps (layout/dtype prep only)."""
    x = np.ascontiguousarray(x, dtype=np.float32)
    centroids = np.ascontiguousarray(centroids, dtype=np.float32)
    labels = np.ascontiguousarray(labels)
    bias = np.ascontiguousarray(bias, dtype=np.float32)

    # x: [16, 256, 56, 56] -> bf16, pad to [58, 58], de-interleave columns.
    xb = x.astype(ml_dtypes.bfloat16)
    xp = np.zeros((16, 256, HP, HP), dtype=ml_dtypes.bfloat16)
    xp[:, :, 1:1 + H, 1:1 + W] = xb
    xde = np.stack([xp[:, :, :, 0::2], xp[:, :, :, 1::2]], axis=2)
    # xde: [16, 256, 2, 58, 29]

    lab8 = labels.astype(np.int8)

    in_maps = []
    for c in range(8):
        b, g = c // 2, c % 2
        xsh = xde[4 * b: 4 * b + 4].reshape(N_IMG_PER_CORE, N_KT, 128, X_FREE)
        lg = lab8[256 * g: 256 * g + 256]
        lg = lg.reshape(N_MT, 128, N_KT, 128, 3, 3)
        lg = lg.transpose(2, 3, 0, 4, 5, 1)
        lg = np.ascontiguousarray(lg).reshape(N_KT, 128, LAB_FREE)
        bg = bias[g].reshape(N_MT, 128)
        in_maps.append({
            "x": np.ascontiguousarray(xsh),
            "labels": lg,
            "centroids": centroids,
            "bias": np.ascontiguousarray(bg),
        })
    return in_maps


def run(x, centroids, labels, bias, trace=False, trace_cores=None):
    nc = _get_nc()
    in_maps = make_in_maps(x, centroids, labels, bias)
    res = run_bass_kernel_spmd(nc, in_maps, list(range(8)), trace=trace,
                               trace_cores=trace_cores)
    out0 = np.empty((16, 256, H, W), dtype=np.float32)
    out1 = np.empty((16, 256, H, W), dtype=np.float32)
    for c in range(8):
        b, g = c // 2, c % 2
        o = np.asarray(res.results[c]["out"]).astype(np.float32)
        o = o.reshape(N_IMG_PER_CORE, 2, 128, 2, H, Q)
        o = o.transpose(0, 1, 2, 4, 5, 3).reshape(N_IMG_PER_CORE, 256, H, W)
        (out0 if g == 0 else out1)[4 * b: 4 * b + 4] = o
    return (out0, out1), res


def kernel(x, centroids, labels, bias):
    (out0, out1), _ = run(x, centroids, labels, bias, trace=False)
    return (out0, out1)


# revision 7
# speedup vs baseline: 1.0454x; 1.0454x over previous
"""Trainium2 Bass kernel for nn_Conv2dShareQ (vq_codebook) — 1D Winograd F(2,3).

Sharding: 4-way batch x 2-way conv group per core (as before).

The 3x3 conv uses Winograd F(2,3) along W only (direct along H and input
channels), cutting PE streamed columns to 2/3 of direct:

  per (ic, row r) and output column pair q:  d_k = xpad[ic, r, 2q+k]
      V0 = d0-d2, V1 = d1+d2, V2 = d2-d1, V3 = d1-d3        (input transform)
  per (oc, ic, ky), g_k = w[oc, ic, ky, k]:
      U0 = g0, U1 = (g0+g1+g2)/2, U2 = (g0-g1+g2)/2, U3 = g2 (weight transform)
  M_xi[oc, h, q] = sum_{ic, ky} U_xi[oc, ic, ky] * V_xi[ic, h+ky, q]   (PE)
  y[oc, h, 2q]   = M0 + M1 + M2      (bias folded into M1's eviction)
  y[oc, h, 2q+1] = M1 - M2 - M3

Engine placement: gather + U on DVE; V on GPSIMD (de-interleaved host layout
makes all four planes plain tensor_tensor ops); M on PE (4 PSUM banks per
(im, mt, h-tile) set, 6 matmuls each); M eviction on ACT (PSUM -> bf16 SBUF);
y assembly 4 tensor_tensor per set on DVE (GPSIMD for the last image).
Output bf16, upcast on host.
"""

import sys

for _p in ("/opt/trn_rl_repo", "/root/.axon_site/_ro/trn_rl_repo"):
    if _p not in sys.path:
        sys.path.append(_p)

import numpy as np
import ml_dtypes

import concourse.bass as bass
import concourse.mybir as mybir
from concourse.tile import TileContext, ScopedClock
from concourse.tile_scheduler import N_PROCS
from bass_rust import VectorClock
from concourse.bass_utils import run_bass_kernel_spmd

F32 = mybir.dt.float32
BF16 = mybir.dt.bfloat16
I8 = mybir.dt.int8

N_IMG_PER_CORE = 4
N_KT = 2
N_MT = 2
N_OFF = 9
H = W = 56
HP = 58                       # padded rows
Q = W // 2                    # 28 output column pairs
QP = Q + 1                    # 29 de-interleaved columns per parity
HW = H * W
N_CENT = 16
CH = N_OFF * 128              # 1152
LAB_FREE = N_MT * CH          # 2304
X_FREE = 2 * HP * QP          # 3364
HT_SIZES = (16, 16, 16, 8)    # h-tiles per (im, mt): 4 PSUM banks each


class SplitDrainTileContext(TileContext):
    """Tail drain split one proc per drain: this walrus build rejects CTRL
    instructions carrying more than one sem wait."""

    def _drain_and_barrier(self, tick_clock, wait_clock):
        gc = tick_clock.global_clock
        for p in range(N_PROCS):
            t = gc[p]
            if t <= 0:
                continue
            vec = [t if q == p else 0 for q in range(N_PROCS)]
            d = self.nc.sync.drain()
            wait_clock.add_sem_waits(d.ins, ScopedClock({None: VectorClock(vec)}))
        self.nc.all_engine_barrier()
        assert self.sems is not None
        popped = self.nc._tile_sem_poison_stack.pop()
        assert popped is self._sem_poison
        self.nc.clear_and_free_semaphores(list(self.sems.allocated().values()))
        self.nc.all_engine_barrier()


def _split_multi_waits(nc, limit=1):
    """Hoist excess sem waits onto wait-only EventSemaphore instructions."""
    for f in nc.m.functions:
        for bb in f.blocks:
            out = []
            for ins in bb.instructions:
                si = ins.sync_info
                if si is not None and si.on_wait and len(si.on_wait) > limit:
                    waits = list(si.on_wait)
                    for w in waits[:-limit]:
                        es = mybir.InstEventSemaphore(
                            name=f"waitsplit_{nc.next_id()}", ins=[], outs=[])
                        es.engine = ins.engine
                        es.sync_info = mybir.SyncInfo(on_wait=[w], on_update=[])
                        out.append(es)
                    si.on_wait = waits[-limit:]
                out.append(ins)
            bb.instructions[:] = out


def build_program():
    nc = bass.Bass()

    x_in = nc.dram_tensor("x", [N_IMG_PER_CORE, N_KT, 128, X_FREE], BF16,
                          kind="ExternalInput")
    labels_in = nc.dram_tensor("labels", [N_KT, 128, LAB_FREE], I8,
                               kind="ExternalInput")
    cent_in = nc.dram_tensor("centroids", [N_CENT], F32, kind="ExternalInput")
    bias_in = nc.dram_tensor("bias", [N_MT, 128], F32, kind="ExternalInput")
    out = nc.dram_tensor("out", [N_IMG_PER_CORE, N_MT, 128, HW], BF16,
                         kind="ExternalOutput")

    with SplitDrainTileContext(nc) as tc:
        with (
            tc.tile_pool(name="consts", bufs=1) as consts,
            tc.tile_pool(name="lab_f", bufs=1) as lab_f_pool,
            tc.tile_pool(name="wq", bufs=1) as wq_pool,
            tc.tile_pool(name="u12", bufs=1) as u12_pool,
            tc.tile_pool(name="tbuf", bufs=1) as tbuf_pool,
            tc.tile_pool(name="xbuf", bufs=1) as xbuf_pool,
            tc.tile_pool(name="vbuf", bufs=1) as vbuf_pool,
            tc.tile_pool(name="msb", bufs=3) as msb_pool,
            tc.tile_pool(name="ytmp", bufs=3) as ytmp_pool,
            tc.tile_pool(name="lstage", bufs=2) as lstage_pool,
            tc.tile_pool(name="obuf", bufs=4) as obuf_pool,
            tc.tile_pool(name="psum", bufs=8, space="PSUM") as psum_pool,
        ):
            cent_sb = consts.tile([128, N_CENT], F32)
            cent_bcast = bass.AP(tensor=cent_in[:].tensor, offset=0,
                                 ap=[[0, 128], [1, N_CENT]])
            nc.sync.dma_start(out=cent_sb[:], in_=cent_bcast)

            lab_stage = {}

            def load_labels(mt):
                for kt in range(N_KT):
                    sl = slice(mt * CH, (mt + 1) * CH)
                    li = lstage_pool.tile([128, CH], I8, tag="ls",
                                          name=f"lab_st{mt}_{kt}")
                    nc.sync.dma_start(out=li[0:64, :], in_=labels_in[kt][0:64, sl])
                    nc.sync.dma_start(out=li[64:128, :], in_=labels_in[kt][64:128, sl])
                    lab_stage[(mt, kt)] = li

            load_labels(0)

            # x tiles: [128, 2(parity), 58, 29] bf16; im shares slots by parity
            xs = {}

            def load_x(im, kt):
                xs[(im, kt)] = xbuf_pool.tile(
                    [128, 2, HP, QP], BF16, tag=f"x{im % 2}_{kt}",
                    name=f"x{im}_{kt}")
                nc.sync.dma_start(out=xs[(im, kt)][:], in_=x_in[im, kt])

            load_x(0, 0)
            load_x(0, 1)

            bias_sb = consts.tile([128, N_MT], F32)
            for mt in range(N_MT):
                nc.sync.dma_start(out=bias_sb[:, mt:mt + 1], in_=bias_in[mt, :])

            load_labels(1)
            load_x(1, 0)
            load_x(1, 1)

            # ---- codebook gather (DVE) ----
            lab_f = [lab_f_pool.tile([128, LAB_FREE], BF16, tag=f"lf{kt}",
                                     name=f"lab_f{kt}")
                     for kt in range(N_KT)]
            # wq free layout per kt: [mt, ky, kx, oo]
            wq = [wq_pool.tile([128, N_MT, 3, 3, 128], BF16, tag=f"wq{kt}",
                               name=f"wq{kt}")
                  for kt in range(N_KT)]
            # u12 free layout per kt: [mt, ky, {u1,u2}, oo]
            u12 = [u12_pool.tile([128, N_MT, 3, 2, 128], BF16, tag=f"u{kt}",
                                 name=f"u12_{kt}")
                   for kt in range(N_KT)]

            def gather_chunk(mt, kt):
                sl = slice(mt * CH, (mt + 1) * CH)
                nc.vector.tensor_copy(out=lab_f[kt][:, sl],
                                      in_=lab_stage.pop((mt, kt))[:])
                t = tbuf_pool.tile([128, N_CENT, CH], BF16, tag="t", name="t")
                for v in range(N_CENT):
                    nc.vector.tensor_scalar(
                        out=t[:, v, :], in0=lab_f[kt][:, sl],
                        scalar1=float(v), scalar2=cent_sb[:, v:v + 1],
                        op0=mybir.AluOpType.is_equal,
                        op1=mybir.AluOpType.mult,
                    )
                s8 = tbuf_pool.tile([128, 8, CH], BF16, tag="s8", name="s8")
                s4 = tbuf_pool.tile([128, 4, CH], BF16, tag="s4", name="s4")
                s2 = tbuf_pool.tile([128, 2, CH], BF16, tag="s2", name="s2")
                wq_flat = wq[kt][:].rearrange("p a b c d -> p (a b c d)")
                hh = CH // 2
                for h in range(2):
                    hs = slice(h * hh, (h + 1) * hh)
                    nc.vector.tensor_tensor(out=s8[:, :, hs], in0=t[:, 0:8, hs],
                                            in1=t[:, 8:16, hs],
                                            op=mybir.AluOpType.add)
                for h in range(2):
                    hs = slice(h * hh, (h + 1) * hh)
                    nc.vector.tensor_tensor(out=s4[:, :, hs], in0=s8[:, 0:4, hs],
                                            in1=s8[:, 4:8, hs],
                                            op=mybir.AluOpType.add)
                for h in range(2):
                    hs = slice(h * hh, (h + 1) * hh)
                    nc.vector.tensor_tensor(out=s2[:, :, hs], in0=s4[:, 0:2, hs],
                                            in1=s4[:, 2:4, hs],
                                            op=mybir.AluOpType.add)
                for h in range(2):
                    hs = slice(h * hh, (h + 1) * hh)
                    nc.vector.tensor_tensor(
                        out=wq_flat[:, mt * CH + h * hh: mt * CH + (h + 1) * hh],
                        in0=s2[:, 0, hs], in1=s2[:, 1, hs],
                        op=mybir.AluOpType.add)

            def u_transform(mt, kt):
                """u1 = (g0+g1+g2)/2, u2 = (g0-g1+g2)/2 for all ky of (mt,kt)."""
                g0 = wq[kt][:, mt, :, 0, :]
                g1 = wq[kt][:, mt, :, 1, :]
                g2 = wq[kt][:, mt, :, 2, :]
                st = ytmp_pool.tile([128, 3, 3, 128], BF16, tag="ut", name="ut")
                nc.vector.tensor_tensor(out=st[:, 0], in0=g0, in1=g2,
                                        op=mybir.AluOpType.add)
                nc.vector.tensor_tensor(out=st[:, 1], in0=st[:, 0], in1=g1,
                                        op=mybir.AluOpType.add)
                nc.vector.tensor_tensor(out=st[:, 2], in0=st[:, 0], in1=g1,
                                        op=mybir.AluOpType.subtract)
                nc.vector.tensor_scalar(out=u12[kt][:, mt, :, 0, :],
                                        in0=st[:, 1], scalar1=0.5,
                                        scalar2=None,
                                        op0=mybir.AluOpType.mult,
                                        op1=mybir.AluOpType.bypass)
                nc.vector.tensor_scalar(out=u12[kt][:, mt, :, 1, :],
                                        in0=st[:, 2], scalar1=0.5,
                                        scalar2=None,
                                        op0=mybir.AluOpType.mult,
                                        op1=mybir.AluOpType.bypass)

            for kt in range(N_KT):
                gather_chunk(0, kt)
                u_transform(0, kt)

            # x for im2/im3 (slots shared by parity with im0/im1; DMAs wait
            # for the V transform of the earlier image to release the slot)
            load_x(2, 0)
            load_x(2, 1)
            load_x(3, 0)
            load_x(3, 1)

            # ---- V transform (GPSIMD), one xi-plane per op ----
            vtiles = {}

            def v_planes(im):
                for xi in range(4):
                    for kt in range(N_KT):
                        if (im, kt) not in vtiles:
                            vtiles[(im, kt)] = vbuf_pool.tile(
                                [128, 4, HP, Q], BF16, tag=f"v{im % 2}_{kt}",
                                name=f"v{im}_{kt}")
                        vt = vtiles[(im, kt)]
                        x = xs[(im, kt)]
                        d0 = x[:, 0, :, 0:Q]
                        d2 = x[:, 0, :, 1:Q + 1]
                        d1 = x[:, 1, :, 0:Q]
                        d3 = x[:, 1, :, 1:Q + 1]
                        if xi == 0:
                            nc.vector.tensor_tensor(out=vt[:, 0], in0=d0, in1=d2,
                                                    op=mybir.AluOpType.subtract)
                        elif xi == 1:
                            nc.vector.tensor_tensor(out=vt[:, 1], in0=d1, in1=d2,
                                                    op=mybir.AluOpType.add)
                        elif xi == 2:
                            nc.vector.tensor_tensor(out=vt[:, 2], in0=d2, in1=d1,
                                                    op=mybir.AluOpType.subtract)
                        else:
                            nc.vector.tensor_tensor(out=vt[:, 3], in0=d1, in1=d3,
                                                    op=mybir.AluOpType.subtract)

            # ---- M sets: PE + ACT eviction + y assembly ----
            def m_set(im, mt, ht):
                hr = HT_SIZES[ht]
                h0 = sum(HT_SIZES[:ht])
                n = hr * Q
                ms = [psum_pool.tile([128, HT_SIZES[0], Q], F32, tag="ps",
                                     name="ps") for _ in range(4)]
                for xi in range(4):
                    idx = 0
                    for kt in range(N_KT):
                        for ky in range(3):
                            if xi == 0:
                                lhsT = wq[kt][:, mt, ky, 0, :]
                            elif xi == 3:
                                lhsT = wq[kt][:, mt, ky, 2, :]
                            else:
                                lhsT = u12[kt][:, mt, ky, xi - 1, :]
                            rhs = vtiles[(im, kt)][:, xi, h0 + ky: h0 + ky + hr, :]
                            nc.tensor.matmul(ms[xi][:, 0:hr, :], lhsT, rhs,
                                             start=(idx == 0), stop=(idx == 5))
                            idx += 1
                mb = msb_pool.tile([128, 4, HT_SIZES[0] * Q], BF16, tag="mb",
                                   name="mb")
                for xi in range(4):
                    nc.scalar.activation(
                        out=mb[:, xi, 0:n],
                        in_=ms[xi][:, 0:hr, :].rearrange("p h q -> p (h q)"),
                        func=mybir.ActivationFunctionType.Identity,
                        bias=bias_sb[:, mt:mt + 1] if xi == 1 else 0.0,
                        scale=1.0,
                    )
                # y assembly: even = M0+M1+M2, odd = M1-M2-M3
                # stored as parity-major planes; host interleaves columns
                ob = obuf_pool.tile([128, 2, HT_SIZES[0] * Q], BF16, tag="ob",
                                    name="ob")
                yt = ytmp_pool.tile([128, 2, HT_SIZES[0] * Q], BF16, tag="yt",
                                    name="yt")
                nc.vector.tensor_tensor(out=yt[:, 0, 0:n], in0=mb[:, 0, 0:n],
                                        in1=mb[:, 1, 0:n],
                                        op=mybir.AluOpType.add)
                nc.vector.tensor_tensor(out=ob[:, 0, 0:n], in0=yt[:, 0, 0:n],
                                        in1=mb[:, 2, 0:n],
                                        op=mybir.AluOpType.add)
                nc.vector.tensor_tensor(out=yt[:, 1, 0:n], in0=mb[:, 1, 0:n],
                                        in1=mb[:, 2, 0:n],
                                        op=mybir.AluOpType.subtract)
                nc.vector.tensor_tensor(out=ob[:, 1, 0:n], in0=yt[:, 1, 0:n],
                                        in1=mb[:, 3, 0:n],
                                        op=mybir.AluOpType.subtract)
                # out free layout per (im, mt): [parity(2), 56h, 28q]
                dst = bass.AP(
                    tensor=out[im, mt].tensor,
                    offset=out[im, mt].offset + h0 * Q,
                    ap=[out[im, mt].ap[0], [H * Q, 2], [1, n]])
                nc.sync.dma_start(out=dst, in_=ob[:, :, 0:n])

            def block(im, mt):
                for ht in range(4):
                    m_set(im, mt, ht)

            v_planes(0)
            v_planes(1)
            block(0, 0)
            gather_chunk(1, 0)
            u_transform(1, 0)
            block(1, 0)
            gather_chunk(1, 1)
            u_transform(1, 1)
            block(0, 1)
            v_planes(2)     # reuses im0's slots (released after block(0, 1))
            block(1, 1)
            v_planes(3)     # reuses im1's slots
            block(2, 0)
            block(2, 1)
            block(3, 0)
            block(3, 1)

    _split_multi_waits(nc)
    return nc


_NC_CACHE = None


def _get_nc():
    global _NC_CACHE
    if _NC_CACHE is None:
        _NC_CACHE = build_program()
    return _NC_CACHE


def make_in_maps(x, centroids, labels, bias):
    """Shard full inputs into 8 per-core input maps (layout/dtype prep only)."""
    x = np.ascontiguousarray(x, dtype=np.float32)
    centroids = np.ascontiguousarray(centroids, dtype=np.float32)
    labels = np.ascontiguousarray(labels)
    bias = np.ascontiguousarray(bias, dtype=np.float32)

    # x: [16, 256, 56, 56] -> bf16, pad to [58, 58], de-interleave columns.
    xb = x.astype(ml_dtypes.bfloat16)
    xp = np.zeros((16, 256, HP, HP), dtype=ml_dtypes.bfloat16)
    xp[:, :, 1:1 + H, 1:1 + W] = xb
    xde = np.stack([xp[:, :, :, 0::2], xp[:, :, :, 1::2]], axis=2)
    # xde: [16, 256, 2, 58, 29]

    lab8 = labels.astype(np.int8)

    in_maps = []
    for c in range(8):
        b, g = c // 2, c % 2
        xsh = xde[4 * b: 4 * b + 4].reshape(N_IMG_PER_CORE, N_KT, 128, X_FREE)
        lg = lab8[256 * g: 256 * g + 256]
        lg = lg.reshape(N_MT, 128, N_KT, 128, 3, 3)
        lg = lg.transpose(2, 3, 0, 4, 5, 1)
        lg = np.ascontiguousarray(lg).reshape(N_KT, 128, LAB_FREE)
        bg = bias[g].reshape(N_MT, 128)
        in_maps.append({
            "x": np.ascontiguousarray(xsh),
            "labels": lg,
            "centroids": centroids,
            "bias": np.ascontiguousarray(bg),
        })
    return in_maps


def run(x, centroids, labels, bias, trace=False, trace_cores=None):
    nc = _get_nc()
    in_maps = make_in_maps(x, centroids, labels, bias)
    res = run_bass_kernel_spmd(nc, in_maps, list(range(8)), trace=trace,
                               trace_cores=trace_cores)
    out0 = np.empty((16, 256, H, W), dtype=np.float32)
    out1 = np.empty((16, 256, H, W), dtype=np.float32)
    for c in range(8):
        b, g = c // 2, c % 2
        o = np.asarray(res.results[c]["out"]).astype(np.float32)
        o = o.reshape(N_IMG_PER_CORE, 2, 128, 2, H, Q)
        o = o.transpose(0, 1, 2, 4, 5, 3).reshape(N_IMG_PER_CORE, 256, H, W)
        (out0 if g == 0 else out1)[4 * b: 4 * b + 4] = o
    return (out0, out1), res


def kernel(x, centroids, labels, bias):
    (out0, out1), _ = run(x, centroids, labels, bias, trace=False)
    return (out0, out1)
